# revision 6
# baseline (speedup 1.0000x reference)
"""Trainium2 Bass LSTM cell kernel — fp8 DoubleRow, coalesced streams.

Same math/distribution/precision as kernel_fp8c (w = mu + dw, dw*2^12
e4m3 DoubleRow, broadcast fp32 mean term, two-pass gate split).

Difference: the host lays x and h.T out so that ALL 8 DoubleRow
super-slices of a 512-column batch tile are contiguous per partition
([n*128+p, j, slot, col] order). The kernel streams ONE 1MB DMA per
half (x, h.T) per batch tile instead of 16 x 128KB — 8KB contiguous
per-partition lines, and the PE matmul groups carry 2 DMA semaphore
waits per pass instead of 16.
"""

import os

import numpy as np

os.environ.setdefault("JAX_COMPILATION_CACHE_DIR", "/tmp/jax_cache")
os.environ.setdefault("JAX_PLATFORMS", "axon,cpu")

N_CORES = 8
IN_SIZE = 2048
HIDDEN = 2048
BATCH = 4096
P = 128
NB = 512
G = 4
KSUP = 2 * P
W_SCALE = 2.0**12


def build_lstm_nc(
    in_size, hid_size, shard, batch, nb=NB, reps=1, loop_reps=0, hints=False,
):
    import concourse.bass as bass
    import concourse.tile as tile
    from concourse import bacc, mybir
    from concourse._compat import get_trn_type

    f32 = mybir.dt.float32
    f8 = mybir.dt.float8e4
    DR = mybir.MatmulPerfMode.DoubleRow
    AF = mybir.ActivationFunctionType
    gate_funcs = [AF.Sigmoid, AF.Sigmoid, AF.Tanh, AF.Sigmoid]

    assert shard % P == 0 and in_size % KSUP == 0 and hid_size % KSUP == 0
    assert batch % nb == 0
    m_tiles = shard // P
    njx = in_size // KSUP
    njh = hid_size // KSUP
    nk = njx + njh
    nn = batch // nb
    gw = G * shard

    nc = bacc.Bacc(get_trn_type() or "TRN2", target_bir_lowering=False, debug=False)

    wx_d = nc.dram_tensor("wx", [njx * P, 2, gw], f8, kind="ExternalInput")
    wh_d = nc.dram_tensor("wh", [njh * P, 2, gw], f8, kind="ExternalInput")
    x_d = nc.dram_tensor("x", [nn * P, njx, 2, nb], f8, kind="ExternalInput")
    ht_d = nc.dram_tensor("ht", [nn * P, njh, 2, nb], f8, kind="ExternalInput")
    ct_d = nc.dram_tensor("ct", [shard, batch], f32, kind="ExternalInput")
    b_d = nc.dram_tensor("bias", [P, G * m_tiles], f32, kind="ExternalInput")
    mc_d = nc.dram_tensor("mc", [1, G * batch], f32, kind="ExternalInput")
    ho_d = nc.dram_tensor("h_out", [shard, batch], f32, kind="ExternalOutput")
    co_d = nc.dram_tensor("c_out", [shard, batch], f32, kind="ExternalOutput")

    with tile.TileContext(nc) as tc:
        with (
            tc.tile_pool(name="wpool", bufs=1) as wpool,
            tc.tile_pool(name="spool", bufs=2) as spool,
            tc.tile_pool(name="cpool", bufs=3) as cpool,
            tc.tile_pool(name="gpool", bufs=2) as gpool,
            tc.tile_pool(name="bpool", bufs=1) as bpool,
            tc.tile_pool(name="mcpool", bufs=2) as mcpool,
            tc.tile_pool(name="psum", bufs=1, space=bass.MemorySpace.PSUM) as pspool,
        ):
            w_sb = []
            for j in range(njx):
                wt = wpool.tile([P, 2, gw], f8, tag=f"wx{j}", name=f"wx{j}")
                nc.gpsimd.dma_start(out=wt[:], in_=wx_d[j * P : (j + 1) * P])
                w_sb.append(wt)
            for j in range(njh):
                wt = wpool.tile([P, 2, gw], f8, tag=f"wh{j}", name=f"wh{j}")
                nc.gpsimd.dma_start(out=wt[:], in_=wh_d[j * P : (j + 1) * P])
                w_sb.append(wt)
            bias_sb = bpool.tile([P, G * m_tiles], f32, name="bias_sb")
            nc.gpsimd.dma_start(out=bias_sb[:], in_=b_d[:])
            mc_sb = bpool.tile([1, G * batch], f32, name="mc_sb")
            nc.gpsimd.dma_start(out=mc_sb[:], in_=mc_d[:])

            def emit_half(ps, gates, rhs_of):
                for kk in range(nk):
                    start = kk == 0
                    stop = kk == nk - 1
                    rhs = rhs_of(kk)
                    for m in range(m_tiles):
                        for g in gates:
                            col = g * shard + m * P
                            nc.tensor.matmul(
                                ps[g][m][:],
                                w_sb[kk][:, :, col : col + P],
                                rhs,
                                start=start,
                                stop=stop,
                                perf_mode=DR,
                            )

            def emit_body():
              for rep in range(reps):
               for n in range(nn):
                ncol = slice(n * nb, (n + 1) * nb)
                nrow = slice(n * P, (n + 1) * P)
                ps = [
                    [
                        pspool.tile([P, nb], f32, tag=f"ps{g}_{m}", name=f"ps{g}_{m}_{n}_{rep}")
                        for m in range(m_tiles)
                    ]
                    for g in range(G)
                ]
                bc = []
                for g in range(G):
                    bt = mcpool.tile([P, nb], f32, tag=f"bc{g}", name=f"bc{g}_{n}_{rep}")
                    nc.gpsimd.partition_broadcast(
                        bt[:], mc_sb[0:1, g * batch + n * nb : g * batch + (n + 1) * nb]
                    )
                    bc.append(bt)
                # One coalesced stream DMA per half: all super-slices of
                # this batch tile, 8KB contiguous per partition.
                xt = spool.tile([P, njx, 2, nb], f8, tag="xt", name=f"xt_{n}_{rep}")
                nc.sync.dma_start(out=xt[:], in_=x_d[nrow])
                htt = spool.tile([P, njh, 2, nb], f8, tag="htt", name=f"htt_{n}_{rep}")
                nc.sync.dma_start(out=htt[:], in_=ht_d[nrow])

                def rhs_of(kk):
                    if kk < njx:
                        return xt[:, kk, :, :]
                    return htt[:, kk - njx, :, :]

                ct_t = []
                for m in range(m_tiles):
                    mrow = slice(m * P, (m + 1) * P)
                    ct = cpool.tile([P, nb], f32, tag=f"ct{m}", name=f"ct{m}_{n}_{rep}")
                    nc.gpsimd.dma_start(out=ct[:], in_=ct_d[mrow, ncol])
                    ct_t.append(ct)

                # Pass A: gates i (0), f (1).
                emit_half(ps, (0, 1), rhs_of)
                gt = [[None] * G for _ in range(m_tiles)]
                for m in range(m_tiles):
                    for g in (0, 1):
                        nc.vector.tensor_add(ps[g][m][:], ps[g][m][:], bc[g][:])
                        gs = gpool.tile([P, nb], f32, tag=f"g{g}_{m}", name=f"g{g}_{m}_{n}_{rep}")
                        nc.scalar.activation(
                            gs[:],
                            ps[g][m][:],
                            gate_funcs[g],
                            bias=bias_sb[:, g * m_tiles + m : g * m_tiles + m + 1],
                            scale=1.0 / W_SCALE,
                        )
                        gt[m][g] = gs
                    nc.vector.tensor_mul(gt[m][1][:], gt[m][1][:], ct_t[m][:])

                # Pass B: gates g (2), o (3), reusing the resident stream tiles.
                emit_half(ps, (2, 3), rhs_of)
                for m in range(m_tiles):
                    mrow = slice(m * P, (m + 1) * P)
                    for g in (2, 3):
                        nc.vector.tensor_add(ps[g][m][:], ps[g][m][:], bc[g][:])
                        gs = gpool.tile([P, nb], f32, tag=f"g{g}_{m}", name=f"g{g}_{m}_{n}_{rep}")
                        nc.scalar.activation(
                            gs[:],
                            ps[g][m][:],
                            gate_funcs[g],
                            bias=bias_sb[:, g * m_tiles + m : g * m_tiles + m + 1],
                            scale=1.0 / W_SCALE,
                        )
                        gt[m][g] = gs
                    i_t, fc_t, g_t, o_t = gt[m]
                    nc.vector.tensor_mul(i_t[:], i_t[:], g_t[:])
                    nc.vector.tensor_add(fc_t[:], fc_t[:], i_t[:])
                    nc.scalar.activation(g_t[:], fc_t[:], AF.Tanh)
                    nc.vector.tensor_mul(o_t[:], o_t[:], g_t[:])
                    nc.gpsimd.dma_start(out=co_d[mrow, ncol], in_=fc_t[:])
                    nc.gpsimd.dma_start(out=ho_d[mrow, ncol], in_=o_t[:])
                del ps

            if loop_reps > 0:
                ET = mybir.EngineType
                hint = (
                    (ET.PE, ET.SP, ET.Activation, ET.DVE, ET.Pool) if hints else ()
                )
                with tc.For_i(0, loop_reps, 1, hint_engines=hint):
                    emit_body()
            else:
                emit_body()

    nc.compile()
    return nc


def build_timing_nc(loop_reps):
    return build_lstm_nc(
        IN_SIZE, HIDDEN, HIDDEN // N_CORES, BATCH, loop_reps=loop_reps, hints=False
    )


_NC_CACHE = {}


def _get_nc(key, *args):
    if key not in _NC_CACHE:
        _NC_CACHE[key] = build_lstm_nc(*args)
    return _NC_CACHE[key]


def prepare_inputs(
    inputs, h, c,
    w_ii, w_if, w_ig, w_io,
    w_hi, w_hf, w_hg, w_ho,
    b_ii, b_hi, b_if, b_hf, b_ig, b_hg, b_io, b_ho,
    n_cores=N_CORES,
):
    """Host-side prep: per-core input maps for the SPMD kernel."""
    import ml_dtypes

    f8 = ml_dtypes.float8_e4m3

    in_size, batch = inputs.shape
    hid = h.shape[1]
    shard = hid // n_cores
    m_tiles = shard // P
    njx = in_size // KSUP
    njh = hid // KSUP
    nn = batch // NB

    x = np.asarray(inputs, dtype=np.float32)
    ht = np.ascontiguousarray(np.asarray(h).T, dtype=np.float32)
    ct = np.ascontiguousarray(np.asarray(c).T, dtype=np.float32)
    s_x = x.sum(axis=0)
    s_h = ht.sum(axis=0)

    def tile_stream(a, nj):
        """[rows, batch] fp32 -> fp8 [n*P + p, j, 2, NB] coalesced layout."""
        a8 = a.astype(f8)
        t = a8.reshape(nj, 2, P, nn, NB).transpose(3, 2, 0, 1, 4)
        return np.ascontiguousarray(t.reshape(nn * P, nj, 2, NB))

    x_t = tile_stream(x, njx)
    ht_t = tile_stream(ht, njh)

    w_in = [w_ii, w_if, w_ig, w_io]
    w_hid = [w_hi, w_hf, w_hg, w_ho]
    biases = [b_ii + b_hi, b_if + b_hf, b_ig + b_hg, b_io + b_ho]

    wxT = [np.ascontiguousarray(np.asarray(w).T, dtype=np.float32) for w in w_in]
    whT = [np.ascontiguousarray(np.asarray(w).T, dtype=np.float32) for w in w_hid]

    def pack_w(wT_shard, mu, nj):
        dw = ((wT_shard - mu) * W_SCALE).astype(f8)
        return dw.reshape(nj, 2, P, dw.shape[1]).transpose(0, 2, 1, 3)

    in_maps = []
    for s in range(n_cores):
        rows = slice(s * shard, (s + 1) * shard)
        mux = [np.float32(w[:, rows].mean()) for w in wxT]
        muh = [np.float32(w[:, rows].mean()) for w in whT]
        wx_s = np.concatenate(
            [pack_w(w[:, rows], mu, njx) for w, mu in zip(wxT, mux)], axis=3
        )
        wh_s = np.concatenate(
            [pack_w(w[:, rows], mu, njh) for w, mu in zip(whT, muh)], axis=3
        )
        mc = np.concatenate(
            [(mux[g] * s_x + muh[g] * s_h) * W_SCALE for g in range(G)]
        ).astype(np.float32)[None, :]
        b_cols = []
        for g in range(G):
            bg = np.asarray(biases[g], dtype=np.float32).reshape(-1)[rows]
            for m in range(m_tiles):
                b_cols.append(bg[m * P : (m + 1) * P])
        bias_s = np.ascontiguousarray(np.stack(b_cols, axis=1), dtype=np.float32)
        in_maps.append(
            {
                "wx": np.ascontiguousarray(wx_s.reshape(njx * P, 2, G * shard)),
                "wh": np.ascontiguousarray(wh_s.reshape(njh * P, 2, G * shard)),
                "x": x_t,
                "ht": ht_t,
                "ct": np.ascontiguousarray(ct[rows, :]),
                "bias": bias_s,
                "mc": np.ascontiguousarray(mc),
            }
        )
    return in_maps


def run_spmd(nc, in_maps, **kwargs):
    from concourse.bass_utils import run_bass_kernel_spmd

    return run_bass_kernel_spmd(nc, in_maps, core_ids=list(range(len(in_maps))), **kwargs)


def assemble_outputs(results):
    ht_next = np.concatenate([r["h_out"] for r in results], axis=0)
    ct_next = np.concatenate([r["c_out"] for r in results], axis=0)
    return ht_next.T, ct_next.T


def kernel(**inputs):
    in_maps = prepare_inputs(**{k: np.asarray(v) for k, v in inputs.items()})
    in_size, batch = inputs["inputs"].shape
    hid = inputs["h"].shape[1]
    shard = hid // N_CORES
    nc = _get_nc((in_size, hid, shard, batch), in_size, hid, shard, batch)
    last_err = None
    for _ in range(3):  # the axon transport transiently flakes ~1 in 4 runs
        try:
            res = run_spmd(nc, in_maps)
            return assemble_outputs(res.results)
        except Exception as e:  # noqa: BLE001
            last_err = e
    raise last_err


# revision 7
# speedup vs baseline: 1.3227x; 1.3227x over previous
"""Trainium2 Bass LSTM cell kernel — fp8 DoubleRow, coalesced streams v2.

kernel_fp8e plus: 3-deep stream double-buffering (two batch tiles of
DMA prefetch slack) and c.T loads moved to the sync queue so the
GpSimd queue (broadcasts + output stores) never delays them behind
data-dependent output DMAs.

Same math/distribution/precision as kernel_fp8c (w = mu + dw, dw*2^12
e4m3 DoubleRow, broadcast fp32 mean term, two-pass gate split).

Difference: the host lays x and h.T out so that ALL 8 DoubleRow
super-slices of a 512-column batch tile are contiguous per partition
([n*128+p, j, slot, col] order). The kernel streams ONE 1MB DMA per
half (x, h.T) per batch tile instead of 16 x 128KB — 8KB contiguous
per-partition lines, and the PE matmul groups carry 2 DMA semaphore
waits per pass instead of 16.
"""

import os

import numpy as np

os.environ.setdefault("JAX_COMPILATION_CACHE_DIR", "/tmp/jax_cache")
os.environ.setdefault("JAX_PLATFORMS", "axon,cpu")

N_CORES = 8
IN_SIZE = 2048
HIDDEN = 2048
BATCH = 4096
P = 128
NB = 512
G = 4
KSUP = 2 * P
W_SCALE = 2.0**12


def build_lstm_nc(
    in_size, hid_size, shard, batch, nb=NB, reps=1, loop_reps=0, hints=False,
):
    import concourse.bass as bass
    import concourse.tile as tile
    from concourse import bacc, mybir
    from concourse._compat import get_trn_type

    f32 = mybir.dt.float32
    f8 = mybir.dt.float8e4
    DR = mybir.MatmulPerfMode.DoubleRow
    AF = mybir.ActivationFunctionType
    gate_funcs = [AF.Sigmoid, AF.Sigmoid, AF.Tanh, AF.Sigmoid]

    assert shard % P == 0 and in_size % KSUP == 0 and hid_size % KSUP == 0
    assert batch % nb == 0
    m_tiles = shard // P
    njx = in_size // KSUP
    njh = hid_size // KSUP
    nk = njx + njh
    nn = batch // nb
    gw = G * shard

    nc = bacc.Bacc(get_trn_type() or "TRN2", target_bir_lowering=False, debug=False)

    wx_d = nc.dram_tensor("wx", [njx * P, 2, gw], f8, kind="ExternalInput")
    wh_d = nc.dram_tensor("wh", [njh * P, 2, gw], f8, kind="ExternalInput")
    x_d = nc.dram_tensor("x", [nn * P, njx, 2, nb], f8, kind="ExternalInput")
    ht_d = nc.dram_tensor("ht", [nn * P, njh, 2, nb], f8, kind="ExternalInput")
    ct_d = nc.dram_tensor("ct", [shard, batch], f32, kind="ExternalInput")
    b_d = nc.dram_tensor("bias", [P, G * m_tiles], f32, kind="ExternalInput")
    mc_d = nc.dram_tensor("mc", [1, G * batch], f32, kind="ExternalInput")
    ho_d = nc.dram_tensor("h_out", [shard, batch], f32, kind="ExternalOutput")
    co_d = nc.dram_tensor("c_out", [shard, batch], f32, kind="ExternalOutput")

    with tile.TileContext(nc) as tc:
        with (
            tc.tile_pool(name="wpool", bufs=1) as wpool,
            tc.tile_pool(name="spool", bufs=3) as spool,
            tc.tile_pool(name="cpool", bufs=3) as cpool,
            tc.tile_pool(name="gpool", bufs=2) as gpool,
            tc.tile_pool(name="bpool", bufs=1) as bpool,
            tc.tile_pool(name="mcpool", bufs=2) as mcpool,
            tc.tile_pool(name="psum", bufs=1, space=bass.MemorySpace.PSUM) as pspool,
        ):
            w_sb = []
            for j in range(njx):
                wt = wpool.tile([P, 2, gw], f8, tag=f"wx{j}", name=f"wx{j}")
                nc.gpsimd.dma_start(out=wt[:], in_=wx_d[j * P : (j + 1) * P])
                w_sb.append(wt)
            for j in range(njh):
                wt = wpool.tile([P, 2, gw], f8, tag=f"wh{j}", name=f"wh{j}")
                nc.gpsimd.dma_start(out=wt[:], in_=wh_d[j * P : (j + 1) * P])
                w_sb.append(wt)
            bias_sb = bpool.tile([P, G * m_tiles], f32, name="bias_sb")
            nc.gpsimd.dma_start(out=bias_sb[:], in_=b_d[:])
            mc_sb = bpool.tile([1, G * batch], f32, name="mc_sb")
            nc.gpsimd.dma_start(out=mc_sb[:], in_=mc_d[:])

            def emit_half(ps, gates, rhs_of):
                for kk in range(nk):
                    start = kk == 0
                    stop = kk == nk - 1
                    rhs = rhs_of(kk)
                    for m in range(m_tiles):
                        for g in gates:
                            col = g * shard + m * P
                            nc.tensor.matmul(
                                ps[g][m][:],
                                w_sb[kk][:, :, col : col + P],
                                rhs,
                                start=start,
                                stop=stop,
                                perf_mode=DR,
                            )

            def emit_body():
              for rep in range(reps):
               for n in range(nn):
                ncol = slice(n * nb, (n + 1) * nb)
                nrow = slice(n * P, (n + 1) * P)
                ps = [
                    [
                        pspool.tile([P, nb], f32, tag=f"ps{g}_{m}", name=f"ps{g}_{m}_{n}_{rep}")
                        for m in range(m_tiles)
                    ]
                    for g in range(G)
                ]
                bc = []
                for g in range(G):
                    bt = mcpool.tile([P, nb], f32, tag=f"bc{g}", name=f"bc{g}_{n}_{rep}")
                    nc.gpsimd.partition_broadcast(
                        bt[:], mc_sb[0:1, g * batch + n * nb : g * batch + (n + 1) * nb]
                    )
                    bc.append(bt)
                # One coalesced stream DMA per half: all super-slices of
                # this batch tile, 8KB contiguous per partition.
                xt = spool.tile([P, njx, 2, nb], f8, tag="xt", name=f"xt_{n}_{rep}")
                nc.sync.dma_start(out=xt[:], in_=x_d[nrow])
                htt = spool.tile([P, njh, 2, nb], f8, tag="htt", name=f"htt_{n}_{rep}")
                nc.sync.dma_start(out=htt[:], in_=ht_d[nrow])

                def rhs_of(kk):
                    if kk < njx:
                        return xt[:, kk, :, :]
                    return htt[:, kk - njx, :, :]

                ct_t = []
                for m in range(m_tiles):
                    mrow = slice(m * P, (m + 1) * P)
                    ct = cpool.tile([P, nb], f32, tag=f"ct{m}", name=f"ct{m}_{n}_{rep}")
                    nc.sync.dma_start(out=ct[:], in_=ct_d[mrow, ncol])
                    ct_t.append(ct)

                # Pass A: gates i (0), f (1).
                emit_half(ps, (0, 1), rhs_of)
                gt = [[None] * G for _ in range(m_tiles)]
                for m in range(m_tiles):
                    for g in (0, 1):
                        nc.vector.tensor_add(ps[g][m][:], ps[g][m][:], bc[g][:])
                        gs = gpool.tile([P, nb], f32, tag=f"g{g}_{m}", name=f"g{g}_{m}_{n}_{rep}")
                        nc.scalar.activation(
                            gs[:],
                            ps[g][m][:],
                            gate_funcs[g],
                            bias=bias_sb[:, g * m_tiles + m : g * m_tiles + m + 1],
                            scale=1.0 / W_SCALE,
                        )
                        gt[m][g] = gs
                    nc.vector.tensor_mul(gt[m][1][:], gt[m][1][:], ct_t[m][:])

                # Pass B: gates g (2), o (3), reusing the resident stream tiles.
                emit_half(ps, (2, 3), rhs_of)
                for m in range(m_tiles):
                    mrow = slice(m * P, (m + 1) * P)
                    for g in (2, 3):
                        nc.vector.tensor_add(ps[g][m][:], ps[g][m][:], bc[g][:])
                        gs = gpool.tile([P, nb], f32, tag=f"g{g}_{m}", name=f"g{g}_{m}_{n}_{rep}")
                        nc.scalar.activation(
                            gs[:],
                            ps[g][m][:],
                            gate_funcs[g],
                            bias=bias_sb[:, g * m_tiles + m : g * m_tiles + m + 1],
                            scale=1.0 / W_SCALE,
                        )
                        gt[m][g] = gs
                    i_t, fc_t, g_t, o_t = gt[m]
                    nc.vector.tensor_mul(i_t[:], i_t[:], g_t[:])
                    nc.vector.tensor_add(fc_t[:], fc_t[:], i_t[:])
                    nc.scalar.activation(g_t[:], fc_t[:], AF.Tanh)
                    nc.vector.tensor_mul(o_t[:], o_t[:], g_t[:])
                    nc.gpsimd.dma_start(out=co_d[mrow, ncol], in_=fc_t[:])
                    nc.gpsimd.dma_start(out=ho_d[mrow, ncol], in_=o_t[:])
                del ps

            if loop_reps > 0:
                ET = mybir.EngineType
                hint = (
                    (ET.PE, ET.SP, ET.Activation, ET.DVE, ET.Pool) if hints else ()
                )
                with tc.For_i(0, loop_reps, 1, hint_engines=hint):
                    emit_body()
            else:
                emit_body()

    nc.compile()
    return nc


def build_timing_nc(loop_reps):
    return build_lstm_nc(
        IN_SIZE, HIDDEN, HIDDEN // N_CORES, BATCH, loop_reps=loop_reps, hints=False
    )


_NC_CACHE = {}


def _get_nc(key, *args):
    if key not in _NC_CACHE:
        _NC_CACHE[key] = build_lstm_nc(*args)
    return _NC_CACHE[key]


def prepare_inputs(
    inputs, h, c,
    w_ii, w_if, w_ig, w_io,
    w_hi, w_hf, w_hg, w_ho,
    b_ii, b_hi, b_if, b_hf, b_ig, b_hg, b_io, b_ho,
    n_cores=N_CORES,
):
    """Host-side prep: per-core input maps for the SPMD kernel."""
    import ml_dtypes

    f8 = ml_dtypes.float8_e4m3

    in_size, batch = inputs.shape
    hid = h.shape[1]
    shard = hid // n_cores
    m_tiles = shard // P
    njx = in_size // KSUP
    njh = hid // KSUP
    nn = batch // NB

    x = np.asarray(inputs, dtype=np.float32)
    ht = np.ascontiguousarray(np.asarray(h).T, dtype=np.float32)
    ct = np.ascontiguousarray(np.asarray(c).T, dtype=np.float32)
    s_x = x.sum(axis=0)
    s_h = ht.sum(axis=0)

    def tile_stream(a, nj):
        """[rows, batch] fp32 -> fp8 [n*P + p, j, 2, NB] coalesced layout."""
        a8 = a.astype(f8)
        t = a8.reshape(nj, 2, P, nn, NB).transpose(3, 2, 0, 1, 4)
        return np.ascontiguousarray(t.reshape(nn * P, nj, 2, NB))

    x_t = tile_stream(x, njx)
    ht_t = tile_stream(ht, njh)

    w_in = [w_ii, w_if, w_ig, w_io]
    w_hid = [w_hi, w_hf, w_hg, w_ho]
    biases = [b_ii + b_hi, b_if + b_hf, b_ig + b_hg, b_io + b_ho]

    wxT = [np.ascontiguousarray(np.asarray(w).T, dtype=np.float32) for w in w_in]
    whT = [np.ascontiguousarray(np.asarray(w).T, dtype=np.float32) for w in w_hid]

    def pack_w(wT_shard, mu, nj):
        dw = ((wT_shard - mu) * W_SCALE).astype(f8)
        return dw.reshape(nj, 2, P, dw.shape[1]).transpose(0, 2, 1, 3)

    in_maps = []
    for s in range(n_cores):
        rows = slice(s * shard, (s + 1) * shard)
        mux = [np.float32(w[:, rows].mean()) for w in wxT]
        muh = [np.float32(w[:, rows].mean()) for w in whT]
        wx_s = np.concatenate(
            [pack_w(w[:, rows], mu, njx) for w, mu in zip(wxT, mux)], axis=3
        )
        wh_s = np.concatenate(
            [pack_w(w[:, rows], mu, njh) for w, mu in zip(whT, muh)], axis=3
        )
        mc = np.concatenate(
            [(mux[g] * s_x + muh[g] * s_h) * W_SCALE for g in range(G)]
        ).astype(np.float32)[None, :]
        b_cols = []
        for g in range(G):
            bg = np.asarray(biases[g], dtype=np.float32).reshape(-1)[rows]
            for m in range(m_tiles):
                b_cols.append(bg[m * P : (m + 1) * P])
        bias_s = np.ascontiguousarray(np.stack(b_cols, axis=1), dtype=np.float32)
        in_maps.append(
            {
                "wx": np.ascontiguousarray(wx_s.reshape(njx * P, 2, G * shard)),
                "wh": np.ascontiguousarray(wh_s.reshape(njh * P, 2, G * shard)),
                "x": x_t,
                "ht": ht_t,
                "ct": np.ascontiguousarray(ct[rows, :]),
                "bias": bias_s,
                "mc": np.ascontiguousarray(mc),
            }
        )
    return in_maps


def run_spmd(nc, in_maps, **kwargs):
    from concourse.bass_utils import run_bass_kernel_spmd

    return run_bass_kernel_spmd(nc, in_maps, core_ids=list(range(len(in_maps))), **kwargs)


def assemble_outputs(results):
    ht_next = np.concatenate([r["h_out"] for r in results], axis=0)
    ct_next = np.concatenate([r["c_out"] for r in results], axis=0)
    return ht_next.T, ct_next.T


def kernel(**inputs):
    in_maps = prepare_inputs(**{k: np.asarray(v) for k, v in inputs.items()})
    in_size, batch = inputs["inputs"].shape
    hid = inputs["h"].shape[1]
    shard = hid // N_CORES
    nc = _get_nc((in_size, hid, shard, batch), in_size, hid, shard, batch)
    last_err = None
    for _ in range(3):  # the axon transport transiently flakes ~1 in 4 runs
        try:
            res = run_spmd(nc, in_maps)
            return assemble_outputs(res.results)
        except Exception as e:  # noqa: BLE001
            last_err = e
    raise last_err
